# revision 1
# baseline (speedup 1.0000x reference)
"""DifferentialAttention Trainium2 kernel (8 NeuronCores, SPMD).

Sharding: data-parallel over batch B=4, tensor-parallel over heads
(2 cores per batch element, 8 heads each).  Each core computes the
partial projection output for its 8 heads; the host sums the two
bf16 partials per batch element in f32 and adds b_proj.

Per-core pipeline (bf16 matmuls, fp32 PSUM):
  1. QKV^T = W_slice^T.T @ x^T            -> [channels, n] layout
  2. V via PE with swapped operands        -> [keys, ch|1] layout
  3. scores S^T[keys, n] per (head, half) with 4-way row groups
  4. exp on ACT (scale=1/8 folded), bf16 out; ACT runs ONLY exp in
     steady state - it is the binding engine (~123us of exp)
  5. PV with stationary-E / moving-[V|1]: out[n, 65] accumulated over
     key chunks; denominator lands per-PARTITION (column 64)
  6. combine on DVE: per-partition reciprocal + scalar_tensor_tensor
     (o1*r1 + o2*(-lam*r2)) -> oc[n, head, ch]
  7. PE transpose (vs identity) oc -> oT[ch, n]; proj = oT.T @ Wp
"""

import sys

sys.path.insert(0, "/opt/trn_rl_repo")

import numpy as np
import ml_dtypes

B, N, C, H, HD = 4, 1024, 1024, 16, 64
LAMBDA_INIT = 0.8
BF16 = ml_dtypes.bfloat16

_PROG_CACHE = {}


def _build_program(loop_n=1, dma_outside=False, skip=(), debug=False):
    key = ("nc", loop_n, dma_outside, tuple(skip), debug)
    if key in _PROG_CACHE:
        return _PROG_CACHE[key]

    import concourse.mybir as mybir
    import concourse.tile as tile
    from concourse import bacc

    f32 = mybir.dt.float32
    b16 = mybir.dt.bfloat16
    Exp = mybir.ActivationFunctionType.Exp
    MUL = mybir.AluOpType.mult
    ADD = mybir.AluOpType.add

    nc = bacc.Bacc(None)

    # host layouts are partition-major so each DMA is one large transfer
    x_d = nc.dram_tensor("xT", [128, 8, N], b16, kind="ExternalInput")
    # wqkv columns reordered: block j'=2*hp+t (t=0 q, t=1 k), v at 1024:1536
    wqkv_d = nc.dram_tensor("wqkv", [128, 8, 1536], b16, kind="ExternalInput")
    wp_d = nc.dram_tensor("wp", [128, 4, C], b16, kind="ExternalInput")
    neglam_d = nc.dram_tensor("neglam", [128, 1], f32, kind="ExternalInput")
    ident_d = nc.dram_tensor("ident", [128, 128], b16, kind="ExternalInput")
    out_d = nc.dram_tensor("out", [8, 128, C], b16, kind="ExternalOutput")
    if debug:
        dbg_qkvT = nc.dram_tensor("dbg_qkvT", [128, 8, N], b16, kind="ExternalOutput")
        dbg_vsb = nc.dram_tensor("dbg_vsb", [128, 8, 8, 65], b16, kind="ExternalOutput")
        dbg_oc = nc.dram_tensor("dbg_oc", [128, 8, 8, 64], b16, kind="ExternalOutput")
        dbg_oT = nc.dram_tensor("dbg_oT", [128, 4, N], b16, kind="ExternalOutput")

    with tile.TileContext(nc) as tc:
        with (
            tc.tile_pool(name="io", bufs=1) as iopool,
            tc.tile_pool(name="work", bufs=4) as wpool,
            tc.tile_pool(name="esb", bufs=10) as epool,
            tc.tile_pool(name="pS", bufs=2, space="PSUM") as pS,
            tc.tile_pool(name="pO", bufs=4, space="PSUM") as pO,
        ):
            xT = iopool.tile([128, 8, N], b16)
            wqkv = iopool.tile([128, 8, 1536], b16)
            wp = iopool.tile([128, 4, C], b16)
            neglam = iopool.tile([128, 1], f32)
            ident = iopool.tile([128, 128], b16)
            # qkvT chunk j=2*hp+t: partitions 0-63 even head d0..63,
            # 64-127 odd head d0..63
            qkvT = iopool.tile([128, 8, N], b16)
            # V in [keys, channels] layout; col 64 of each head = ones
            vsb = iopool.tile([128, 8, 8, 65], b16)
            # combined attention out: [n-part, global n-chunk, head, ch]
            oc = iopool.tile([128, 8, 8, 64], b16)
            # transposed for proj: [ch-part, hp, n]
            oT = iopool.tile([128, 4, N], b16)

            nc.gpsimd.memset(vsb[:, :, :, 64:65], 1.0)
            if "attn" in skip:
                nc.gpsimd.memset(oT[:], 0.0)

            def dma_inputs():
                # transfers serialize globally in issue order; order by need
                for h4 in range(4):
                    nc.sync.dma_start(
                        xT[:, 2 * h4 : 2 * h4 + 2, :], x_d[:, 2 * h4 : 2 * h4 + 2, :]
                    )
                nc.scalar.dma_start(wqkv[:, :, 0:256], wqkv_d[:, :, 0:256])
                nc.gpsimd.dma_start(ident[:], ident_d[:])
                nc.gpsimd.dma_start(neglam[:], neglam_d[:])
                nc.scalar.dma_start(wqkv[:, :, 1024:1536], wqkv_d[:, :, 1024:1536])
                for hp in range(1, 4):
                    c0 = hp * 256
                    nc.scalar.dma_start(
                        wqkv[:, :, c0 : c0 + 256], wqkv_d[:, :, c0 : c0 + 256]
                    )
                nc.sync.dma_start(wp[:], wp_d[:])

            def qkv_psum():
                t = pS.tile([128, 2, 512], f32, tag="s", name="qkvps")
                return t[:, 0, :]

            if dma_outside:
                dma_inputs()
            assert loop_n == 1
            if not dma_outside:
                dma_inputs()

            # ---------------- QKV projection units ---------------------
            def emit_qkv_unit(hp, t, nh, on_act=True):
                # one [128,512] q/k projection chunk -> qkvT[:, 2hp+t, nh]
                evac = nc.scalar.copy if on_act else nc.vector.tensor_copy
                j = 2 * hp + t
                ps = qkv_psum()
                for cc in range(8):
                    nc.tensor.matmul(
                        ps[:],
                        wqkv[:, cc, j * 128 : (j + 1) * 128],
                        xT[:, cc, nh * 512 : (nh + 1) * 512],
                        start=(cc == 0),
                        stop=(cc == 7),
                    )
                evac(qkvT[:, j, nh * 512 : (nh + 1) * 512], ps[:])

            def emit_v_unit(mc):
                # v -> [keys, channels] layout (operands swapped); DVE evac
                ps = qkv_psum()
                for cc in range(8):
                    nc.tensor.matmul(
                        ps[:],
                        xT[:, cc, mc * 128 : (mc + 1) * 128],
                        wqkv[:, cc, 1024:1536],
                        start=(cc == 0),
                        stop=(cc == 7),
                    )
                nc.vector.tensor_copy(
                    vsb[:, mc, :, 0:64], ps.rearrange("p (g d) -> p g d", g=8)
                )

            # phase A: q/k for head-pair 0 (ACT evac - no exps yet)
            if "qkv" not in skip:
                for t in range(2):
                    for nh in range(2):
                        emit_qkv_unit(0, t, nh, on_act=True)

            # ---------------- attention sweeps --------------------------
            # combo ci: 0=(even,h1) 1=(odd,h1) 2=(even,h2) 3=(odd,h2)
            # score row group rg for ci: [0, 2, 1, 3][ci]; parity = ci%2
            RG = [0, 2, 1, 3]
            LAG = 3

            def emit_scores_exp(hp, nh, mc):
                # 4 score matmuls + 2 exps; returns e tiles [g0, g1]
                cur = []
                for g in range(2):
                    s_ps = pS.tile([128, 2, 512], f32, tag="s")
                    for i in range(2):
                        rg = RG[2 * g + i]
                        nc.tensor.matmul(
                            s_ps[:, i, :],
                            qkvT[
                                32 * rg : 32 * rg + 32,
                                2 * hp + 1,
                                mc * 128 : (mc + 1) * 128,
                            ],
                            qkvT[
                                32 * rg : 32 * rg + 32,
                                2 * hp,
                                nh * 512 : (nh + 1) * 512,
                            ],
                            start=True,
                            stop=True,
                            tile_position=(32 * rg, 0),
                        )
                    e_sb = epool.tile([128, 2, 512], b16, tag="e")
                    nc.scalar.activation(e_sb[:], s_ps[:], Exp, scale=0.125)
                    cur.append(e_sb)
                return cur

            def emit_pv(hp, etiles, o_tiles, mc):
                # stationary-E PV: 16 matmuls [128n, 65] accumulating over mc
                # one start/stop per PSUM bank: start=True zeroes the whole
                # 2KB bank, so only the bank's first write may set it
                for g in range(2):
                    for i in range(2):
                        ci = 2 * g + i
                        par = ci % 2
                        for nsub in range(4):
                            nc.tensor.matmul(
                                o_tiles[ci][:, nsub, :],
                                etiles[g][:, i, nsub * 128 : (nsub + 1) * 128],
                                vsb[:, mc, 2 * hp + par, :],
                                start=(mc == 0 and nsub == 0),
                                stop=(mc == 7 and nsub == 3),
                                skip_group_check=True,
                            )

            def emit_combine(hp, nh, o_tiles):
                # per-partition reciprocals, -lam fold, fused combine on DVE
                r = wpool.tile([128, 4, 4, 1], f32, tag="r")
                for ci in range(4):
                    nc.vector.reciprocal(r[:, ci], o_tiles[ci][:, :, 64:65])
                nc.vector.tensor_scalar_mul(r[:, 2:4], r[:, 2:4], neglam[:])
                for par in range(2):
                    ci1, ci2 = par, 2 + par
                    for nsub in range(4):
                        gn = nh * 4 + nsub
                        t = wpool.tile([128, 64], f32, tag=f"t{par}")
                        nc.vector.tensor_scalar_mul(
                            t[:], o_tiles[ci2][:, nsub, 0:64], r[:, ci2, nsub]
                        )
                        nc.vector.scalar_tensor_tensor(
                            oc[:, gn, 2 * hp + par, :],
                            o_tiles[ci1][:, nsub, 0:64],
                            r[:, ci1, nsub],
                            t[:],
                            MUL,
                            ADD,
                        )

            def emit_transposes(hp, nh):
                # oc[n, 2 heads, 64] -> oT[128 ch, n] via PE transpose
                for nsub in range(4):
                    gn = nh * 4 + nsub
                    trb = pO.tile([128, 128], b16, tag="o", name="tr")
                    nc.tensor.matmul(
                        trb[:],
                        oc[:, gn, 2 * hp : 2 * hp + 2, :],
                        ident[:],
                        is_transpose=True,
                    )
                    nc.vector.tensor_copy(
                        oT[:, hp, gn * 128 : (gn + 1) * 128], trb[:]
                    )

            if "attn" not in skip:
                qkv_queue = []
                pending = None  # (hp, nh, etile-list, o_tiles)
                for sweep in range(8):
                    hp, nh = sweep // 2, sweep % 2
                    if nh == 0 and hp < 3 and "qkv" not in skip:
                        qkv_queue = [
                            (hp + 1, t, nh2) for t in range(2) for nh2 in range(2)
                        ]
                    o_tiles = None
                    edeque = []
                    for mc in range(8):
                        edeque.append(emit_scores_exp(hp, nh, mc))
                        if sweep == 0 and "qkv" not in skip:
                            emit_v_unit(mc)
                        if mc == 1 and pending is not None:
                            # previous sweep's combine + transposes, placed
                            # after this sweep's pipeline is warmed up
                            emit_combine(*pending)
                            emit_transposes(pending[0], pending[1])
                            pending = None
                        if mc >= LAG:
                            if o_tiles is None:
                                # allocated after the previous sweep's
                                # transposes so the slot ring hands off
                                # ci -> tr -> next ci without deadlock
                                o_tiles = [
                                    pO.tile(
                                        [128, 4, 65], f32, tag="o",
                                        name=f"o{sweep}_{ci}",
                                    )
                                    for ci in range(4)
                                ]
                            emit_pv(hp, edeque[mc - LAG], o_tiles, mc - LAG)
                        if mc in (3, 6) and qkv_queue:
                            emit_qkv_unit(*qkv_queue.pop(0), on_act=False)
                    for mc in range(max(0, 8 - LAG), 8):
                        if o_tiles is None:
                            o_tiles = [
                                pO.tile(
                                    [128, 4, 65], f32, tag="o",
                                    name=f"o{sweep}_{ci}",
                                )
                                for ci in range(4)
                            ]
                        emit_pv(hp, edeque[mc], o_tiles, mc)
                    pending = (hp, nh, o_tiles)

                if pending is not None:
                    emit_combine(*pending)
                    emit_transposes(pending[0], pending[1])
                    pending = None

            if debug:
                nc.sync.dma_start(dbg_qkvT[:], qkvT[:])
                nc.sync.dma_start(dbg_vsb[:], vsb[:])
                nc.sync.dma_start(dbg_oc[:], oc[:])
                nc.sync.dma_start(dbg_oT[:], oT[:])

            # ---------------- output projection -------------------------
            if "proj" not in skip:
                for ncc in range(8):
                    for jh in range(2):
                        ps = qkv_psum()
                        for ci in range(4):
                            nc.tensor.matmul(
                                ps[:],
                                oT[:, ci, ncc * 128 : (ncc + 1) * 128],
                                wp[:, ci, jh * 512 : (jh + 1) * 512],
                                start=(ci == 0),
                                stop=(ci == 3),
                            )
                        osb = wpool.tile([128, 512], b16, tag="osb")
                        nc.scalar.copy(osb[:], ps[:])
                        nc.sync.dma_start(
                            out_d[ncc, :, jh * 512 : (jh + 1) * 512], osb[:]
                        )

    nc.compile()
    _PROG_CACHE[key] = nc
    return nc


def _prep_core_inputs(x, W_qkv, W_proj, neg_lam):
    """Host-side shard prep. Returns in_maps for the 8 cores."""
    W4 = np.asarray(W_qkv, np.float32).reshape(3, H, HD, C)
    ident = np.eye(128, dtype=np.float32).astype(BF16)
    in_maps = []
    for core in range(8):
        b, hg = divmod(core, 2)
        xT = (
            np.ascontiguousarray(np.asarray(x[b], np.float32).T)
            .reshape(8, 128, N)
            .transpose(1, 0, 2)
            .astype(BF16)
        )
        wsl = W4[:, hg * 8 : (hg + 1) * 8]  # [3, 8 heads, 64, 1024]
        # columns [t(3), head(8), d(64)]; reorder q/k to j'=2*hp+t blocks
        Wcols = np.ascontiguousarray(wsl.transpose(3, 0, 1, 2).reshape(C, 1536))
        Wnew = np.empty_like(Wcols)
        for hp in range(4):
            for t in range(2):
                src = t * 512 + hp * 128
                dst = (2 * hp + t) * 128
                Wnew[:, dst : dst + 128] = Wcols[:, src : src + 128]
        Wnew[:, 1024:1536] = Wcols[:, 1024:1536]
        wqkv = Wnew.reshape(8, 128, 1536).transpose(1, 0, 2).astype(BF16)
        wp = (
            np.ascontiguousarray(
                np.asarray(W_proj, np.float32)[:, hg * 512 : (hg + 1) * 512].T
            )
            .reshape(4, 128, C)
            .transpose(1, 0, 2)
            .astype(BF16)
        )
        in_maps.append(
            {
                "xT": np.ascontiguousarray(xT),
                "wqkv": np.ascontiguousarray(wqkv),
                "wp": np.ascontiguousarray(wp),
                "neglam": np.full((128, 1), neg_lam, np.float32),
                "ident": ident,
            }
        )
    return in_maps


def kernel(x, W_qkv, W_proj, b_proj, lambda_q1, lambda_k1, lambda_q2, lambda_k2):
    from concourse.bass_utils import run_bass_kernel_spmd

    lq1 = np.asarray(lambda_q1, np.float64)
    lk1 = np.asarray(lambda_k1, np.float64)
    lq2 = np.asarray(lambda_q2, np.float64)
    lk2 = np.asarray(lambda_k2, np.float64)
    lam = float(np.mean(np.exp(lq1 * lk1) - np.exp(lq2 * lk2) + LAMBDA_INIT))

    nc = _build_program()
    in_maps = _prep_core_inputs(x, W_qkv, W_proj, -lam)
    res = run_bass_kernel_spmd(nc, in_maps, core_ids=list(range(8)))
    _PROG_CACHE["last_result"] = res

    bp = np.asarray(b_proj, np.float32)
    out = np.empty((B, N, C), np.float32)
    for b in range(B):
        p0 = res.results[2 * b]["out"].astype(np.float32).reshape(N, C)
        p1 = res.results[2 * b + 1]["out"].astype(np.float32).reshape(N, C)
        out[b] = p0 + p1 + bp[None, :]
    return out



# revision 3
# speedup vs baseline: 1.0354x; 1.0354x over previous
"""DifferentialAttention Trainium2 kernel (8 NeuronCores, SPMD).

Sharding: data-parallel over batch B=4, tensor-parallel over heads
(2 cores per batch element, 8 heads each).  Each core computes the
partial projection output for its 8 heads; the host sums the two
bf16 partials per batch element in f32 and adds b_proj.

Per-core pipeline (bf16 matmuls, fp32 PSUM), 8 sweeps of
(head-pair hp, 512-col n-half nh):
  1. QKV^T = W_slice^T.T @ x^T            -> [channels, n] layout
  2. V via PE with swapped operands        -> [keys, ch|1] layout
  3. scores S^T[keys, n] per (head, half) with 4-way row groups;
     PSUM writes bank-aligned (tile_position requires it)
  4. exp on ACT (scale=1/8 folded), bf16 out; ACT runs ONLY exp -
     it is the binding engine (~133us); all evacuations are on DVE
  5. PV stationary-E into a single 3-bank tile: 16 chunks of
     [128 n, 65] (g = 4*ci + nsub) packed 7/7/2 per bank; col 64 of
     each chunk accumulates the softmax denominator
  6. combine on DVE: strided reciprocals + scalar_tensor_tensor
     (o1*r1 + o2*(-lam*r2)) -> oc[n, nsub, par, ch]
  7. oc -> oT[ch, n] via DMA transpose (XBAR) - no PE/PSUM cost
  8. proj = oT.T @ Wp: n-half 0 interleaved into the last sweep,
     n-half 1 pipelined per-nsub after the final combine
qkv/V/proj units share a 1-bank PSUM pool (disjoint in time).
"""

import sys

sys.path.insert(0, "/opt/trn_rl_repo")

import numpy as np
import ml_dtypes

B, N, C, H, HD = 4, 1024, 1024, 16, 64
LAMBDA_INIT = 0.8
BF16 = ml_dtypes.bfloat16

_PROG_CACHE = {}

LAG = 3
# combo ci=2g+i: 0=(even,h1) 1=(odd,h1) 2=(even,h2) 3=(odd,h2)
# score row group rg for ci (partition range of the half in qkvT)
RG = [0, 2, 1, 3]
# PV chunk g=4*ci+nsub -> (bank, 65-col slot): 7/7/2 packing
GB = [(g // 7, g % 7) for g in range(16)]
G_START = (0, 7, 14)  # first chunk written in each bank (zeroes it)
G_STOP = (6, 13, 15)  # last chunk written in each bank


def _build_program(debug=False):
    key = ("nc", debug)
    if key in _PROG_CACHE:
        return _PROG_CACHE[key]

    import concourse.mybir as mybir
    import concourse.tile as tile
    from concourse import bacc

    f32 = mybir.dt.float32
    b16 = mybir.dt.bfloat16
    Exp = mybir.ActivationFunctionType.Exp
    MUL = mybir.AluOpType.mult
    ADD = mybir.AluOpType.add

    nc = bacc.Bacc(None)

    # host layouts are partition-major so each DMA is one large transfer
    x_d = nc.dram_tensor("xT", [128, 8, N], b16, kind="ExternalInput")
    # wqkv columns reordered: block j'=2*hp+t (t=0 q, t=1 k), v at 1024:1536
    wqkv_d = nc.dram_tensor("wqkv", [128, 8, 1536], b16, kind="ExternalInput")
    wp_d = nc.dram_tensor("wp", [128, 4, C], b16, kind="ExternalInput")
    neglam_d = nc.dram_tensor("neglam", [128, 1], f32, kind="ExternalInput")
    out_d = nc.dram_tensor("out", [8, 128, C], b16, kind="ExternalOutput")
    if debug:
        dbg_qkvT = nc.dram_tensor("dbg_qkvT", [128, 8, N], b16, kind="ExternalOutput")
        dbg_vsb = nc.dram_tensor("dbg_vsb", [128, 8, 8, 65], b16, kind="ExternalOutput")
        dbg_oc = nc.dram_tensor("dbg_oc", [128, 8, 4, 2, 64], b16, kind="ExternalOutput")
        dbg_oT = nc.dram_tensor("dbg_oT", [128, 4, N], b16, kind="ExternalOutput")

    with tile.TileContext(nc) as tc:
        with (
            tc.tile_pool(name="io", bufs=1) as iopool,
            tc.tile_pool(name="work", bufs=4) as wpool,
            tc.tile_pool(name="esb", bufs=10) as epool,
            tc.tile_pool(name="ocp", bufs=2) as ocpool,
            tc.tile_pool(name="pS", bufs=2, space="PSUM") as pS,
            tc.tile_pool(name="pO", bufs=1, space="PSUM") as pO,
            tc.tile_pool(name="pP", bufs=1, space="PSUM") as pP,
        ):
            xT = iopool.tile([128, 8, N], b16)
            wqkv = iopool.tile([128, 8, 1536], b16)
            wp = iopool.tile([128, 4, C], b16)
            neglam = iopool.tile([128, 1], f32)
            # qkvT chunk j=2*hp+t: partitions 0-63 even head d0..63,
            # 64-127 odd head d0..63
            qkvT = iopool.tile([128, 8, N], b16)
            # V in [keys, channels] layout; col 64 of each head = ones
            vsb = iopool.tile([128, 8, 8, 65], b16)
            # transposed attention out for proj: [ch-part, hp, n]
            oT = iopool.tile([128, 4, N], b16)
            if debug:
                dbg_oc_t = iopool.tile([128, 8, 4, 2, 64], b16)

            nc.gpsimd.memset(vsb[:, :, :, 64:65], 1.0)

            # ---------------- DMA in (finest-need-first order) ----------
            # x first half (keys/n 0..511, all cc) gates the first qkv units
            nc.sync.dma_start(xT[:, :, 0:512], x_d[:, :, 0:512])
            nc.scalar.dma_start(wqkv[:, :, 0:256], wqkv_d[:, :, 0:256])
            nc.sync.dma_start(xT[:, :, 512:1024], x_d[:, :, 512:1024])
            nc.gpsimd.dma_start(neglam[:], neglam_d[:])
            nc.scalar.dma_start(wqkv[:, :, 1024:1536], wqkv_d[:, :, 1024:1536])
            for hp in range(1, 4):
                c0 = hp * 256
                nc.scalar.dma_start(wqkv[:, :, c0 : c0 + 256], wqkv_d[:, :, c0 : c0 + 256])
            nc.gpsimd.dma_start(wp[:], wp_d[:])

            # ---------------- filler units ------------------------------
            def emit_qkv_unit(hp, t, nh):
                # one [128,512] q/k projection chunk -> qkvT[:, 2hp+t, nh]
                j = 2 * hp + t
                ps = pP.tile([128, 512], f32, tag="p", name=f"qkv{j}_{nh}")
                for cc in range(8):
                    nc.tensor.matmul(
                        ps[:],
                        wqkv[:, cc, j * 128 : (j + 1) * 128],
                        xT[:, cc, nh * 512 : (nh + 1) * 512],
                        start=(cc == 0),
                        stop=(cc == 7),
                    )
                nc.vector.tensor_copy(qkvT[:, j, nh * 512 : (nh + 1) * 512], ps[:])

            def emit_v_unit(mc):
                # v -> [keys, channels] layout (operands swapped); DVE evac
                ps = pP.tile([128, 512], f32, tag="p", name=f"v{mc}")
                for cc in range(8):
                    nc.tensor.matmul(
                        ps[:],
                        xT[:, cc, mc * 128 : (mc + 1) * 128],
                        wqkv[:, cc, 1024:1536],
                        start=(cc == 0),
                        stop=(cc == 7),
                    )
                nc.vector.tensor_copy(
                    vsb[:, mc, :, 0:64], ps.rearrange("p (g d) -> p g d", g=8)
                )

            def emit_proj_unit(ncc, jh, pool):
                # out[ncc n-chunk, jh 512 out-ch] = oT.T @ wp, K=512 (4 ci)
                ps = pool.tile([128, 512], f32, tag=pool.name[1].lower(),
                               name=f"proj{ncc}_{jh}")
                for ci in range(4):
                    nc.tensor.matmul(
                        ps[:],
                        oT[:, ci, ncc * 128 : (ncc + 1) * 128],
                        wp[:, ci, jh * 512 : (jh + 1) * 512],
                        start=(ci == 0),
                        stop=(ci == 3),
                    )
                osb = wpool.tile([128, 512], b16, tag="osb")
                nc.vector.tensor_copy(osb[:], ps[:])
                nc.sync.dma_start(out_d[ncc, :, jh * 512 : (jh + 1) * 512], osb[:])

            # ---------------- attention pieces --------------------------
            def emit_scores_exp(hp, nh, mc):
                # 4 score matmuls + 2 exps; returns e tiles [g0, g1]
                cur = []
                for g in range(2):
                    s_ps = pS.tile([128, 2, 512], f32, tag="s")
                    for i in range(2):
                        rg = RG[2 * g + i]
                        nc.tensor.matmul(
                            s_ps[:, i, :],
                            qkvT[
                                32 * rg : 32 * rg + 32,
                                2 * hp + 1,
                                mc * 128 : (mc + 1) * 128,
                            ],
                            qkvT[
                                32 * rg : 32 * rg + 32,
                                2 * hp,
                                nh * 512 : (nh + 1) * 512,
                            ],
                            start=True,
                            stop=True,
                            tile_position=(32 * rg, 0),
                        )
                    e_sb = epool.tile([128, 2, 512], b16, tag="e")
                    nc.scalar.activation(e_sb[:], s_ps[:], Exp, scale=0.125)
                    cur.append(e_sb)
                return cur

            def emit_pv(hp, etiles, o, mc):
                # stationary-E PV into the 3-bank packed tile o [128,3,512]
                # chunk g=4*ci+nsub at (bank g//7, col (g%7)*65); start=True
                # zeroes the whole bank so only its first chunk may set it
                for ci in range(4):
                    par = ci % 2
                    for nsub in range(4):
                        g = 4 * ci + nsub
                        b, sl = GB[g]
                        nc.tensor.matmul(
                            o[:, b, sl * 65 : sl * 65 + 65],
                            etiles[ci // 2][:, par, nsub * 128 : (nsub + 1) * 128],
                            vsb[:, mc, 2 * hp + par, :],
                            start=(mc == 0 and g in G_START),
                            stop=(mc == 7 and g in G_STOP),
                            skip_group_check=True,
                        )

            def emit_combine_chunk(hp, o, r, oc, nsub):
                # one nsub (128 n cols): 2 parities, fused on DVE
                for par in range(2):
                    g1 = 4 * par + nsub
                    g2 = 8 + 4 * par + nsub
                    b1, s1 = GB[g1]
                    b2, s2 = GB[g2]
                    t = wpool.tile([128, 64], f32, tag=f"t{par}")
                    nc.vector.tensor_scalar_mul(
                        t[:], o[:, b2, s2 * 65 : s2 * 65 + 64], r[:, b2, s2]
                    )
                    nc.vector.scalar_tensor_tensor(
                        oc[:, nsub, par, :],
                        o[:, b1, s1 * 65 : s1 * 65 + 64],
                        r[:, b1, s1],
                        t[:],
                        MUL,
                        ADD,
                    )

            def emit_recips(o):
                # per-partition reciprocals of the stride-65 denominators
                r = wpool.tile([128, 3, 7, 1], f32, tag="r")
                for b, cnt in ((0, 7), (1, 7), (2, 2)):
                    vb = o[:, b, 0 : cnt * 65].rearrange("p (g c) -> p g c", c=65)
                    nc.vector.reciprocal(r[:, b, 0:cnt], vb[:, :, 64:65])
                # -lam fold on the h2 chunks (g 8..15)
                nc.vector.tensor_scalar_mul(r[:, 1, 1:7], r[:, 1, 1:7], neglam[:])
                nc.vector.tensor_scalar_mul(r[:, 2, 0:2], r[:, 2, 0:2], neglam[:])
                return r

            def emit_combine(hp, nh, o):
                r = emit_recips(o)
                oc = ocpool.tile([128, 4, 2, 64], b16, tag="oc")
                for nsub in range(4):
                    emit_combine_chunk(hp, o, r, oc, nsub)
                if debug:
                    nc.vector.tensor_copy(dbg_oc_t[:, 2 * hp + nh], oc[:])
                return oc

            def emit_transpose(hp, nh, oc, nsub):
                # oc[128 n, par, 64ch] -> oT[128 ch, n] via DMA xbar
                n0 = nh * 512 + nsub * 128
                nc.sync.dma_start_transpose(oT[:, hp, n0 : n0 + 128], oc[:, nsub])

            # ---------------- filler schedule ---------------------------
            filler = {}

            def add(slot, fn, *args):
                filler.setdefault(slot, []).append((fn, args))

            # sweep 0 (hp0,nh0): V units + rest of hp0 qkv
            add((0, 0), emit_v_unit, 0)
            add((0, 1), emit_v_unit, 1)
            add((0, 2), emit_qkv_unit, 0, 1, 1)  # k hp0 keys 512-1023 (mc4+)
            add((0, 3), emit_v_unit, 2)
            add((0, 4), emit_v_unit, 3)
            add((0, 5), emit_qkv_unit, 0, 0, 1)  # q hp0 n 512+ (sweep 1)
            add((0, 6), emit_v_unit, 4)
            add((0, 7), emit_v_unit, 5)
            add((0, 7), emit_v_unit, 6)
            add((0, 7), emit_v_unit, 7)
            # sweep 1: hp1 qkv (needed by sweep 2)
            add((1, 1), emit_qkv_unit, 1, 1, 0)
            add((1, 3), emit_qkv_unit, 1, 1, 1)
            add((1, 5), emit_qkv_unit, 1, 0, 0)
            add((1, 7), emit_qkv_unit, 1, 0, 1)
            # sweeps 2-3: hp2 (needed by sweep 4)
            add((2, 2), emit_qkv_unit, 2, 1, 0)
            add((2, 6), emit_qkv_unit, 2, 1, 1)
            add((3, 2), emit_qkv_unit, 2, 0, 0)
            add((3, 6), emit_qkv_unit, 2, 0, 1)
            # sweeps 4-5: hp3 (needed by sweep 6)
            add((4, 2), emit_qkv_unit, 3, 1, 0)
            add((4, 6), emit_qkv_unit, 3, 1, 1)
            add((5, 2), emit_qkv_unit, 3, 0, 0)
            add((5, 6), emit_qkv_unit, 3, 0, 1)
            # sweep 7: proj for n-half 0 (oT[:, :, 0:512] complete after the
            # combine+transposes of sweep 6, emitted at (7,1))
            add((7, 2), emit_proj_unit, 0, 0, pP)
            add((7, 3), emit_proj_unit, 0, 1, pP)
            add((7, 4), emit_proj_unit, 1, 0, pP)
            add((7, 5), emit_proj_unit, 1, 1, pP)
            add((7, 6), emit_proj_unit, 2, 0, pP)
            add((7, 6), emit_proj_unit, 2, 1, pP)
            add((7, 7), emit_proj_unit, 3, 0, pP)
            add((7, 7), emit_proj_unit, 3, 1, pP)

            # qkv for the first sweep must precede it
            emit_qkv_unit(0, 1, 0)  # k hp0 keys 0-511
            emit_qkv_unit(0, 0, 0)  # q hp0 n 0-511

            # ---------------- main pipeline -----------------------------
            pending = None  # (hp, nh, o_tile)
            for s in range(8):
                hp, nh = s // 2, s % 2
                o = None
                edeque = []
                for mc in range(8):
                    edeque.append(emit_scores_exp(hp, nh, mc))
                    for fn, args in filler.get((s, mc), ()):
                        fn(*args)
                    if mc == 1 and pending is not None:
                        oc = emit_combine(*pending)
                        for nsub in range(4):
                            emit_transpose(pending[0], pending[1], oc, nsub)
                        pending = None
                    if mc >= LAG:
                        if o is None:
                            o = pO.tile([128, 3, 512], f32, tag="o", name=f"o{s}")
                        emit_pv(hp, edeque[mc - LAG], o, mc - LAG)
                for mc in range(8 - LAG, 8):
                    emit_pv(hp, edeque[mc], o, mc)
                pending = (hp, nh, o)

            # final sweep: chunked combine -> transpose -> proj pipeline
            hp, nh, o = pending
            r = emit_recips(o)
            oc = ocpool.tile([128, 4, 2, 64], b16, tag="oc")
            for nsub in range(4):
                emit_combine_chunk(hp, o, r, oc, nsub)
                emit_transpose(hp, nh, oc, nsub)
                emit_proj_unit(4 + nsub, 0, pS if nsub % 2 else pP)
                emit_proj_unit(4 + nsub, 1, pS if nsub % 2 else pP)
            if debug:
                nc.vector.tensor_copy(dbg_oc_t[:, 2 * hp + nh], oc[:])

            if debug:
                nc.sync.dma_start(dbg_qkvT[:], qkvT[:])
                nc.sync.dma_start(dbg_vsb[:], vsb[:])
                nc.sync.dma_start(dbg_oc[:], dbg_oc_t[:])
                nc.sync.dma_start(dbg_oT[:], oT[:])

    nc.compile()
    _PROG_CACHE[key] = nc
    return nc


def _prep_core_inputs(x, W_qkv, W_proj, neg_lam):
    """Host-side shard prep. Returns in_maps for the 8 cores."""
    W4 = np.asarray(W_qkv, np.float32).reshape(3, H, HD, C)
    in_maps = []
    for core in range(8):
        b, hg = divmod(core, 2)
        xT = (
            np.ascontiguousarray(np.asarray(x[b], np.float32).T)
            .reshape(8, 128, N)
            .transpose(1, 0, 2)
            .astype(BF16)
        )
        wsl = W4[:, hg * 8 : (hg + 1) * 8]  # [3, 8 heads, 64, 1024]
        # columns [t(3), head(8), d(64)]; reorder q/k to j'=2*hp+t blocks
        Wcols = np.ascontiguousarray(wsl.transpose(3, 0, 1, 2).reshape(C, 1536))
        Wnew = np.empty_like(Wcols)
        for hp in range(4):
            for t in range(2):
                src = t * 512 + hp * 128
                dst = (2 * hp + t) * 128
                Wnew[:, dst : dst + 128] = Wcols[:, src : src + 128]
        Wnew[:, 1024:1536] = Wcols[:, 1024:1536]
        wqkv = Wnew.reshape(8, 128, 1536).transpose(1, 0, 2).astype(BF16)
        wp = (
            np.ascontiguousarray(
                np.asarray(W_proj, np.float32)[:, hg * 512 : (hg + 1) * 512].T
            )
            .reshape(4, 128, C)
            .transpose(1, 0, 2)
            .astype(BF16)
        )
        in_maps.append(
            {
                "xT": np.ascontiguousarray(xT),
                "wqkv": np.ascontiguousarray(wqkv),
                "wp": np.ascontiguousarray(wp),
                "neglam": np.full((128, 1), neg_lam, np.float32),
            }
        )
    return in_maps


def kernel(x, W_qkv, W_proj, b_proj, lambda_q1, lambda_k1, lambda_q2, lambda_k2):
    from concourse.bass_utils import run_bass_kernel_spmd

    lq1 = np.asarray(lambda_q1, np.float64)
    lk1 = np.asarray(lambda_k1, np.float64)
    lq2 = np.asarray(lambda_q2, np.float64)
    lk2 = np.asarray(lambda_k2, np.float64)
    lam = float(np.mean(np.exp(lq1 * lk1) - np.exp(lq2 * lk2) + LAMBDA_INIT))

    nc = _build_program()
    in_maps = _prep_core_inputs(x, W_qkv, W_proj, -lam)
    res = run_bass_kernel_spmd(nc, in_maps, core_ids=list(range(8)))
    _PROG_CACHE["last_result"] = res

    bp = np.asarray(b_proj, np.float32)
    out = np.empty((B, N, C), np.float32)
    for b in range(B):
        p0 = res.results[2 * b]["out"].astype(np.float32).reshape(N, C)
        p1 = res.results[2 * b + 1]["out"].astype(np.float32).reshape(N, C)
        out[b] = p0 + p1 + bp[None, :]
    return out


# revision 24
# speedup vs baseline: 1.1327x; 1.0940x over previous
"""DifferentialAttention Trainium2 kernel (8 NeuronCores, SPMD).

Sharding: data-parallel over batch B=4, tensor-parallel over heads
(2 cores per batch element, 8 heads each).  Each core computes the
partial projection output for its 8 heads; the host sums the two
bf16 partials per batch element in f32 and adds b_proj.

Per-core pipeline (bf16 matmuls, fp32 PSUM), 8 sweeps of
(head-pair hp, 512-col n-half nh):
  1. QKV^T = W_slice^T.T @ x^T            -> [channels, n] layout
  2. V via PE with swapped operands        -> [keys, ch|1] layout
  3. scores S^T[keys, n] per (head, half) with 4-way row groups;
     PSUM writes bank-aligned (tile_position requires it)
  4. exp on ACT (scale=1/8 folded), bf16 out; ACT runs ONLY exp -
     it is the binding engine (~133us); all evacuations are on DVE
  5. PV stationary-E into a single 3-bank tile: 16 chunks of
     [128 n, 65] (g = 4*ci + nsub) packed 7/7/2 per bank; col 64 of
     each chunk accumulates the softmax denominator
  6. combine on DVE: strided reciprocals + scalar_tensor_tensor
     (o1*r1 + o2*(-lam*r2)) -> oc[n, nsub, par, ch]
  7. oc -> oT[ch, n] via DMA transpose (XBAR) - no PE/PSUM cost
  8. proj = oT.T @ Wp: n-half 0 interleaved into the last sweep,
     n-half 1 pipelined per-nsub after the final combine
qkv/V/proj units share a 1-bank PSUM pool (disjoint in time).
"""

import sys

sys.path.insert(0, "/opt/trn_rl_repo")

import numpy as np
import ml_dtypes

B, N, C, H, HD = 4, 1024, 1024, 16, 64
LAMBDA_INIT = 0.8
BF16 = ml_dtypes.bfloat16

_PROG_CACHE = {}

LAG = 3
# combo ci=2g+i: 0=(even,h1) 1=(odd,h1) 2=(even,h2) 3=(odd,h2)
# score row group rg for ci (partition range of the half in qkvT)
RG = [0, 2, 1, 3]
# PV chunk g=4*ci+nsub -> (bank, 65-col slot): 7/7/2 packing
GB = [(g // 7, g % 7) for g in range(16)]
G_START = (0, 7, 14)  # first chunk written in each bank (zeroes it)
G_STOP = (6, 13, 15)  # last chunk written in each bank


def _build_program(debug=False):
    key = ("nc", debug)
    if key in _PROG_CACHE:
        return _PROG_CACHE[key]

    import concourse.mybir as mybir
    import concourse.tile as tile
    from concourse import bacc

    f32 = mybir.dt.float32
    b16 = mybir.dt.bfloat16
    Exp = mybir.ActivationFunctionType.Exp
    MUL = mybir.AluOpType.mult
    ADD = mybir.AluOpType.add

    nc = bacc.Bacc(None)

    # host layouts are partition-major so each DMA is one large transfer
    x_d = nc.dram_tensor("xT", [128, 8, N], b16, kind="ExternalInput")
    # wqkv columns reordered: block j'=2*hp+t (t=0 q, t=1 k), v at 1024:1536
    wqkv_d = nc.dram_tensor("wqkv", [128, 8, 1536], b16, kind="ExternalInput")
    wp_d = nc.dram_tensor("wp", [128, 4, C], b16, kind="ExternalInput")
    neglam_d = nc.dram_tensor("neglam", [128, 1], f32, kind="ExternalInput")
    ident_d = nc.dram_tensor("ident", [128, 128], b16, kind="ExternalInput")
    out_d = nc.dram_tensor("out", [8, 128, C], b16, kind="ExternalOutput")
    if debug:
        dbg_qkvT = nc.dram_tensor("dbg_qkvT", [128, 8, N], b16, kind="ExternalOutput")
        dbg_vsb = nc.dram_tensor("dbg_vsb", [128, 8, 8, 65], b16, kind="ExternalOutput")
        dbg_oc = nc.dram_tensor("dbg_oc", [128, 8, 4, 2, 64], b16, kind="ExternalOutput")
        dbg_oT = nc.dram_tensor("dbg_oT", [128, 4, N], b16, kind="ExternalOutput")

    with tile.TileContext(nc) as tc:
        with (
            tc.tile_pool(name="io", bufs=1) as iopool,
            tc.tile_pool(name="work", bufs=4) as wpool,
            tc.tile_pool(name="esb", bufs=10) as epool,
            tc.tile_pool(name="ocp", bufs=2) as ocpool,
            tc.tile_pool(name="pS", bufs=2, space="PSUM") as pS,
            tc.tile_pool(name="pO", bufs=1, space="PSUM") as pO,
            tc.tile_pool(name="pP", bufs=1, space="PSUM") as pP,
        ):
            xT = iopool.tile([128, 8, N], b16)
            wqkv = iopool.tile([128, 8, 1536], b16)
            wp = iopool.tile([128, 4, C], b16)
            neglam = iopool.tile([128, 1], f32)
            # qkvT chunk j=2*hp+t: partitions 0-63 even head d0..63,
            # 64-127 odd head d0..63
            qkvT = iopool.tile([128, 8, N], b16)
            # V in [keys, channels] layout; col 64 of each head = ones
            vsb = iopool.tile([128, 8, 8, 65], b16)
            # transposed attention out for proj: [ch-part, hp, n]
            oT = iopool.tile([128, 4, N], b16)
            ident = iopool.tile([128, 128], b16)
            if debug:
                dbg_oc_t = iopool.tile([128, 8, 4, 2, 64], b16)

            warm = iopool.tile([128, 256], b16)
            nc.gpsimd.memset(warm[:], 0.5)
            nc.gpsimd.memset(vsb[:, :, :, 64:65], 1.0)

            # ---------------- DMA in (strict need order) ----------------
            # The shared DMA slot grants waiting transfers in ARBITRARY
            # order, so a late-need transfer that queues early can starve a
            # critical one.  Put everything on the sync queue in exact need
            # order - its ~1.3us per-issue pacing self-throttles - except
            # w[j0,j1] which rides the otherwise-empty scalar queue.
            nc.scalar.dma_start(wqkv[:, :, 0:256], wqkv_d[:, :, 0:256])
            for cc2 in range(4):
                nc.sync.dma_start(
                    xT[:, 2 * cc2 : 2 * cc2 + 2, 0:512],
                    x_d[:, 2 * cc2 : 2 * cc2 + 2, 0:512],
                )
            for h4 in range(2):
                nc.sync.dma_start(
                    xT[:, 4 * h4 : 4 * h4 + 4, 512:1024],
                    x_d[:, 4 * h4 : 4 * h4 + 4, 512:1024],
                )
            nc.sync.dma_start(wqkv[:, :, 1024:1280], wqkv_d[:, :, 1024:1280])
            nc.sync.dma_start(wqkv[:, :, 1280:1536], wqkv_d[:, :, 1280:1536])
            nc.gpsimd.dma_start(neglam[:], neglam_d[:])
            for hp in range(1, 4):
                c0 = hp * 256
                nc.sync.dma_start(wqkv[:, :, c0 : c0 + 256], wqkv_d[:, :, c0 : c0 + 256])
            nc.sync.dma_start(wp[:], wp_d[:])
            nc.sync.dma_start(ident[:], ident_d[:])

            # PE warm-up during the input DMAs: the pstate model needs ~3us
            # of continuous execution to reach full clock, so burn it on
            # dummy matmuls into a single scratch PSUM tile (one slot alloc
            # so the pP ring is not serialized)
            wps = pP.tile([128, 512], f32, tag="p", name="warm")
            for w in range(10):
                nc.tensor.matmul(
                    wps[0:1, 0:256], warm[:, 0:1], warm[:], start=True, stop=True
                )

            # ---------------- filler units ------------------------------
            def emit_qkv_unit(hp, t, nh):
                # one [128,512] q/k projection chunk -> qkvT[:, 2hp+t, nh]
                j = 2 * hp + t
                ps = pP.tile([128, 512], f32, tag="p", name=f"qkv{j}_{nh}")
                for cc in range(8):
                    nc.tensor.matmul(
                        ps[:],
                        wqkv[:, cc, j * 128 : (j + 1) * 128],
                        xT[:, cc, nh * 512 : (nh + 1) * 512],
                        start=(cc == 0),
                        stop=(cc == 7),
                    )
                nc.vector.tensor_copy(qkvT[:, j, nh * 512 : (nh + 1) * 512], ps[:])

            def emit_v_unit(mc):
                # v -> [keys, channels] layout (operands swapped); DVE evac
                ps = pP.tile([128, 512], f32, tag="p", name=f"v{mc}")
                for cc in range(8):
                    nc.tensor.matmul(
                        ps[:],
                        xT[:, cc, mc * 128 : (mc + 1) * 128],
                        wqkv[:, cc, 1024:1536],
                        start=(cc == 0),
                        stop=(cc == 7),
                    )
                nc.vector.tensor_copy(
                    vsb[:, mc, :, 0:64], ps.rearrange("p (g d) -> p g d", g=8)
                )

            def emit_proj_unit(ncc, jh, pool, on_act=False):
                # out[ncc n-chunk, jh 512 out-ch] = oT.T @ wp, K=512 (4 ci)
                # tail units evacuate on ACT (idle once the exps are done)
                ps = pool.tile([128, 512], f32, tag=pool.name[1].lower(),
                               name=f"proj{ncc}_{jh}")
                for ci in range(4):
                    nc.tensor.matmul(
                        ps[:],
                        oT[:, ci, ncc * 128 : (ncc + 1) * 128],
                        wp[:, ci, jh * 512 : (jh + 1) * 512],
                        start=(ci == 0),
                        stop=(ci == 3),
                    )
                osb = wpool.tile([128, 512], b16, tag="osb")
                if on_act:
                    # tail: ACT evac + scalar-queue DMA keeps the sync queue
                    # free for the final transposes' waits
                    nc.scalar.copy(osb[:], ps[:])
                    nc.scalar.dma_start(out_d[ncc, :, jh * 512 : (jh + 1) * 512], osb[:])
                else:
                    nc.vector.tensor_copy(osb[:], ps[:])
                    nc.sync.dma_start(out_d[ncc, :, jh * 512 : (jh + 1) * 512], osb[:])

            # ---------------- attention pieces --------------------------
            def emit_scores_exp(hp, nh, mc):
                # 4 score matmuls + 2 exps; returns e tiles [g0, g1]
                cur = []
                for g in range(2):
                    s_ps = pS.tile([128, 2, 512], f32, tag="s")
                    for i in range(2):
                        rg = RG[2 * g + i]
                        nc.tensor.matmul(
                            s_ps[:, i, :],
                            qkvT[
                                32 * rg : 32 * rg + 32,
                                2 * hp + 1,
                                mc * 128 : (mc + 1) * 128,
                            ],
                            qkvT[
                                32 * rg : 32 * rg + 32,
                                2 * hp,
                                nh * 512 : (nh + 1) * 512,
                            ],
                            start=True,
                            stop=True,
                            tile_position=(32 * rg, 0),
                        )
                    e_sb = epool.tile([128, 2, 512], b16, tag="e")
                    nc.scalar.activation(e_sb[:], s_ps[:], Exp, scale=0.125)
                    cur.append(e_sb)
                return cur

            def emit_pv(hp, etiles, o, mc):
                # stationary-E PV into the 3-bank packed tile o [128,3,512]
                # chunk g=4*ci+nsub at (bank g//7, col (g%7)*65); start=True
                # zeroes the whole bank so only its first chunk may set it
                for ci in range(4):
                    par = ci % 2
                    for nsub in range(4):
                        g = 4 * ci + nsub
                        b, sl = GB[g]
                        nc.tensor.matmul(
                            o[:, b, sl * 65 : sl * 65 + 65],
                            etiles[ci // 2][:, par, nsub * 128 : (nsub + 1) * 128],
                            vsb[:, mc, 2 * hp + par, :],
                            start=(mc == 0 and g in G_START),
                            stop=(mc == 7 and g in G_STOP),
                            skip_group_check=True,
                        )

            def emit_oraw(o):
                # bulk-evacuate the packed accumulator to SBUF (3 copies,
                # one per bank) so the PSUM tile is released fast - the
                # normalization then runs off the critical path from SBUF
                oraw = wpool.tile([128, 16, 65], f32, tag="oraw")
                orv = oraw.rearrange("p g c -> p (g c)")
                for b, cnt in ((0, 7), (1, 7), (2, 2)):
                    nc.vector.tensor_copy(
                        orv[:, b * 455 : b * 455 + cnt * 65], o[:, b, 0 : cnt * 65]
                    )
                return oraw

            def emit_recips(oraw):
                # per-partition reciprocals of the stride-65 denominators
                r = wpool.tile([128, 16, 1], f32, tag="r")
                nc.vector.reciprocal(r[:], oraw[:, :, 64:65])
                # -lam fold on the h2 chunks (g 8..15)
                nc.vector.tensor_scalar_mul(r[:, 8:16], r[:, 8:16], neglam[:])
                return r

            def emit_combine_chunk(hp, oraw, r, oc, nsub):
                # one nsub (128 n cols): 2 parities, fused on DVE, all SBUF
                for par in range(2):
                    g1 = 4 * par + nsub
                    g2 = 8 + 4 * par + nsub
                    t = wpool.tile([128, 64], f32, tag=f"t{par}")
                    nc.vector.tensor_scalar_mul(
                        t[:], oraw[:, g2, 0:64], r[:, g2]
                    )
                    nc.vector.scalar_tensor_tensor(
                        oc[:, nsub, par, :],
                        oraw[:, g1, 0:64],
                        r[:, g1],
                        t[:],
                        MUL,
                        ADD,
                    )

            def emit_combine(hp, nh, o):
                oraw = emit_oraw(o)
                r = emit_recips(oraw)
                oc = ocpool.tile([128, 4, 2, 64], b16, tag="oc")
                for nsub in range(4):
                    emit_combine_chunk(hp, oraw, r, oc, nsub)
                if debug:
                    nc.vector.tensor_copy(dbg_oc_t[:, 2 * hp + nh], oc[:])
                return oc

            def emit_transpose(hp, nh, oc, nsub):
                # oc[128 n, par, 64ch] -> oT[128 ch, n] via DMA xbar
                n0 = nh * 512 + nsub * 128
                nc.sync.dma_start_transpose(oT[:, hp, n0 : n0 + 128], oc[:, nsub])

            def emit_transpose_pe(hp, nh, oc, nsub):
                # final-sweep transpose on PE (PSUM is free, and the ~2.4us
                # DMA-transpose latency would sit on the critical tail)
                n0 = nh * 512 + nsub * 128
                trb = pS.tile([128, 128], b16, tag="s", name="trb")
                nc.tensor.matmul(trb[:], oc[:, nsub], ident[:], is_transpose=True)
                nc.scalar.copy(oT[:, hp, n0 : n0 + 128], trb[:])

            # ---------------- filler schedule ---------------------------
            filler = {}

            def add(slot, fn, *args):
                filler.setdefault(slot, []).append((fn, args))

            # sweep 0 (hp0,nh0): V units + rest of hp0 qkv
            add((0, 0), emit_v_unit, 0)
            add((0, 1), emit_v_unit, 1)
            add((0, 2), emit_qkv_unit, 0, 1, 1)  # k hp0 keys 512-1023 (mc4+)
            add((0, 3), emit_v_unit, 2)
            add((0, 4), emit_v_unit, 3)
            add((0, 5), emit_qkv_unit, 0, 0, 1)  # q hp0 n 512+ (sweep 1)
            add((0, 6), emit_v_unit, 4)
            add((0, 7), emit_v_unit, 5)
            add((0, 7), emit_v_unit, 6)
            add((0, 7), emit_v_unit, 7)
            # sweep 1: hp1 qkv (needed by sweep 2)
            add((1, 1), emit_qkv_unit, 1, 1, 0)
            add((1, 3), emit_qkv_unit, 1, 1, 1)
            add((1, 5), emit_qkv_unit, 1, 0, 0)
            add((1, 7), emit_qkv_unit, 1, 0, 1)
            # sweeps 2-3: hp2 (needed by sweep 4)
            add((2, 2), emit_qkv_unit, 2, 1, 0)
            add((2, 6), emit_qkv_unit, 2, 1, 1)
            add((3, 2), emit_qkv_unit, 2, 0, 0)
            add((3, 6), emit_qkv_unit, 2, 0, 1)
            # sweeps 4-5: hp3 (needed by sweep 6)
            add((4, 2), emit_qkv_unit, 3, 1, 0)
            add((4, 6), emit_qkv_unit, 3, 1, 1)
            add((5, 2), emit_qkv_unit, 3, 0, 0)
            add((5, 6), emit_qkv_unit, 3, 0, 1)
            # sweep 7: proj for n-half 0 (oT[:, :, 0:512] complete after the
            # combine+transposes of sweep 6, emitted at (7,1)); the last two
            # units are held back to fill the final-combine window
            add((7, 2), emit_proj_unit, 0, 0, pP)
            add((7, 4), emit_proj_unit, 0, 1, pP)
            add((7, 6), emit_proj_unit, 1, 0, pP)

            # qkv for the first sweep must precede it
            emit_qkv_unit(0, 1, 0)  # k hp0 keys 0-511
            emit_qkv_unit(0, 0, 0)  # q hp0 n 0-511

            # ---------------- main pipeline -----------------------------
            # software-pipelined across sweep boundaries: the tail PVs of
            # sweep s are emitted after sweep s+1's first scores/exp so the
            # next exp is never behind them on PE; combine lands at mc==1
            pending = None  # (hp, nh, o_tile)
            tailpv = None  # (hp, edeque, o_tile)
            for s in range(8):
                hp, nh = s // 2, s % 2
                o = None
                edeque = []
                for mc in range(8):
                    edeque.append(emit_scores_exp(hp, nh, mc))
                    if mc == 0 and tailpv is not None:
                        for mcp in range(8 - LAG, 8):
                            emit_pv(tailpv[0], tailpv[1][mcp], tailpv[2], mcp)
                        tailpv = None
                    for fn, args in filler.get((s, mc), ()):
                        fn(*args)
                    if mc == 1 and pending is not None:
                        oc = emit_combine(*pending)
                        for nsub in range(4):
                            emit_transpose(pending[0], pending[1], oc, nsub)
                        pending = None
                    if mc >= LAG:
                        if o is None:
                            o = pO.tile([128, 3, 512], f32, tag="o", name=f"o{s}")
                        emit_pv(hp, edeque[mc - LAG], o, mc - LAG)
                pending = (hp, nh, o)
                tailpv = (hp, edeque, o)

            # final sweep: tail PVs, then chunked combine -> transpose ->
            # proj pipeline; two held-back n-half-0 proj units keep PE warm
            hp, nh, o = pending
            for mcp in range(8 - LAG, 8):
                emit_pv(hp, tailpv[1][mcp], o, mcp)
            oraw = emit_oraw(o)
            r = emit_recips(oraw)
            # held-back n-half-0 proj units fill the combine window on PE
            emit_proj_unit(1, 1, pS, on_act=True)
            emit_proj_unit(2, 0, pP, on_act=True)
            emit_proj_unit(2, 1, pS, on_act=True)
            emit_proj_unit(3, 0, pP, on_act=True)
            emit_proj_unit(3, 1, pS, on_act=True)
            oc = ocpool.tile([128, 4, 2, 64], b16, tag="oc")
            for nsub in range(4):
                emit_combine_chunk(hp, oraw, r, oc, nsub)
                emit_transpose_pe(hp, nh, oc, nsub)
                emit_proj_unit(4 + nsub, 0, pP if nsub % 2 else pS, on_act=True)
                emit_proj_unit(4 + nsub, 1, pS if nsub % 2 else pP, on_act=True)
            if debug:
                nc.vector.tensor_copy(dbg_oc_t[:, 2 * hp + nh], oc[:])

            if debug:
                nc.sync.dma_start(dbg_qkvT[:], qkvT[:])
                nc.sync.dma_start(dbg_vsb[:], vsb[:])
                nc.sync.dma_start(dbg_oc[:], dbg_oc_t[:])
                nc.sync.dma_start(dbg_oT[:], oT[:])

    nc.compile()
    _PROG_CACHE[key] = nc
    return nc


def _prep_core_inputs(x, W_qkv, W_proj, neg_lam):
    """Host-side shard prep. Returns in_maps for the 8 cores."""
    W4 = np.asarray(W_qkv, np.float32).reshape(3, H, HD, C)
    in_maps = []
    for core in range(8):
        b, hg = divmod(core, 2)
        xT = (
            np.ascontiguousarray(np.asarray(x[b], np.float32).T)
            .reshape(8, 128, N)
            .transpose(1, 0, 2)
            .astype(BF16)
        )
        wsl = W4[:, hg * 8 : (hg + 1) * 8]  # [3, 8 heads, 64, 1024]
        # columns [t(3), head(8), d(64)]; reorder q/k to j'=2*hp+t blocks
        Wcols = np.ascontiguousarray(wsl.transpose(3, 0, 1, 2).reshape(C, 1536))
        Wnew = np.empty_like(Wcols)
        for hp in range(4):
            for t in range(2):
                src = t * 512 + hp * 128
                dst = (2 * hp + t) * 128
                Wnew[:, dst : dst + 128] = Wcols[:, src : src + 128]
        Wnew[:, 1024:1536] = Wcols[:, 1024:1536]
        wqkv = Wnew.reshape(8, 128, 1536).transpose(1, 0, 2).astype(BF16)
        wp = (
            np.ascontiguousarray(
                np.asarray(W_proj, np.float32)[:, hg * 512 : (hg + 1) * 512].T
            )
            .reshape(4, 128, C)
            .transpose(1, 0, 2)
            .astype(BF16)
        )
        in_maps.append(
            {
                "xT": np.ascontiguousarray(xT),
                "wqkv": np.ascontiguousarray(wqkv),
                "wp": np.ascontiguousarray(wp),
                "neglam": np.full((128, 1), neg_lam, np.float32),
                "ident": np.eye(128, dtype=np.float32).astype(BF16),
            }
        )
    return in_maps


def kernel(x, W_qkv, W_proj, b_proj, lambda_q1, lambda_k1, lambda_q2, lambda_k2):
    from concourse.bass_utils import run_bass_kernel_spmd

    lq1 = np.asarray(lambda_q1, np.float64)
    lk1 = np.asarray(lambda_k1, np.float64)
    lq2 = np.asarray(lambda_q2, np.float64)
    lk2 = np.asarray(lambda_k2, np.float64)
    lam = float(np.mean(np.exp(lq1 * lk1) - np.exp(lq2 * lk2) + LAMBDA_INIT))

    nc = _build_program()
    in_maps = _prep_core_inputs(x, W_qkv, W_proj, -lam)
    res = run_bass_kernel_spmd(nc, in_maps, core_ids=list(range(8)))
    _PROG_CACHE["last_result"] = res

    bp = np.asarray(b_proj, np.float32)
    out = np.empty((B, N, C), np.float32)
    for b in range(B):
        p0 = res.results[2 * b]["out"].astype(np.float32).reshape(N, C)
        p1 = res.results[2 * b + 1]["out"].astype(np.float32).reshape(N, C)
        out[b] = p0 + p1 + bp[None, :]
    return out


# revision 30
# speedup vs baseline: 1.1354x; 1.0024x over previous
"""DifferentialAttention Trainium2 kernel (8 NeuronCores, SPMD).

Sharding: data-parallel over batch B=4, tensor-parallel over heads
(2 cores per batch element, 8 heads each).  Each core computes the
partial projection output for its 8 heads; the host sums the two
bf16 partials per batch element in f32 and adds b_proj.

Per-core pipeline (bf16 matmuls, fp32 PSUM), 8 sweeps of
(head-pair hp, 512-col n-half nh):
  1. QKV^T = W_slice^T.T @ x^T            -> [channels, n] layout
  2. V via PE with swapped operands        -> [keys, ch|1] layout
  3. scores S^T[keys, n] per (head, half) with 4-way row groups;
     PSUM writes bank-aligned (tile_position requires it)
  4. exp on ACT (scale=1/8 folded), bf16 out; ACT runs ONLY exp -
     it is the binding engine (~133us); all evacuations are on DVE
  5. PV stationary-E into a single 3-bank tile: 16 chunks of
     [128 n, 65] (g = 4*ci + nsub) packed 7/7/2 per bank; col 64 of
     each chunk accumulates the softmax denominator
  6. combine on DVE: strided reciprocals + scalar_tensor_tensor
     (o1*r1 + o2*(-lam*r2)) -> oc[n, nsub, par, ch]
  7. oc -> oT[ch, n] via DMA transpose (XBAR) - no PE/PSUM cost
  8. proj = oT.T @ Wp: n-half 0 interleaved into the last sweep,
     n-half 1 pipelined per-nsub after the final combine
qkv/V/proj units share a 1-bank PSUM pool (disjoint in time).
"""

import sys

sys.path.insert(0, "/opt/trn_rl_repo")

import numpy as np
import ml_dtypes

B, N, C, H, HD = 4, 1024, 1024, 16, 64
LAMBDA_INIT = 0.8
BF16 = ml_dtypes.bfloat16

_PROG_CACHE = {}

LAG = 3
# combo ci=2g+i: 0=(even,h1) 1=(odd,h1) 2=(even,h2) 3=(odd,h2)
# score row group rg for ci (partition range of the half in qkvT)
RG = [0, 2, 1, 3]
# PV chunk g=4*ci+nsub -> (bank, 65-col slot): 7/7/2 packing
GB = [(g // 7, g % 7) for g in range(16)]
G_START = (0, 7, 14)  # first chunk written in each bank (zeroes it)
G_STOP = (6, 13, 15)  # last chunk written in each bank


def _build_program(debug=False):
    key = ("nc", debug)
    if key in _PROG_CACHE:
        return _PROG_CACHE[key]

    import concourse.mybir as mybir
    import concourse.tile as tile
    from concourse import bacc

    f32 = mybir.dt.float32
    b16 = mybir.dt.bfloat16
    Exp = mybir.ActivationFunctionType.Exp
    MUL = mybir.AluOpType.mult
    ADD = mybir.AluOpType.add

    nc = bacc.Bacc(None)

    # host layouts are partition-major so each DMA is one large transfer
    x_d = nc.dram_tensor("xT", [128, 8, N], b16, kind="ExternalInput")
    # wqkv columns reordered: block j'=2*hp+t (t=0 q, t=1 k), v at 1024:1536
    wqkv_d = nc.dram_tensor("wqkv", [128, 8, 1536], b16, kind="ExternalInput")
    wp_d = nc.dram_tensor("wp", [128, 4, C], b16, kind="ExternalInput")
    neglam_d = nc.dram_tensor("neglam", [128, 1], f32, kind="ExternalInput")
    ident_d = nc.dram_tensor("ident", [128, 128], b16, kind="ExternalInput")
    out_d = nc.dram_tensor("out", [8, 128, C], b16, kind="ExternalOutput")
    if debug:
        dbg_qkvT = nc.dram_tensor("dbg_qkvT", [128, 8, N], b16, kind="ExternalOutput")
        dbg_vsb = nc.dram_tensor("dbg_vsb", [128, 8, 8, 65], b16, kind="ExternalOutput")
        dbg_oc = nc.dram_tensor("dbg_oc", [128, 8, 4, 2, 64], b16, kind="ExternalOutput")
        dbg_oT = nc.dram_tensor("dbg_oT", [128, 4, N], b16, kind="ExternalOutput")

    with tile.TileContext(nc) as tc:
        with (
            tc.tile_pool(name="io", bufs=1) as iopool,
            tc.tile_pool(name="work", bufs=4) as wpool,
            tc.tile_pool(name="esb", bufs=10) as epool,
            tc.tile_pool(name="ocp", bufs=2) as ocpool,
            tc.tile_pool(name="pS", bufs=2, space="PSUM") as pS,
            tc.tile_pool(name="pO", bufs=1, space="PSUM") as pO,
            tc.tile_pool(name="pP", bufs=1, space="PSUM") as pP,
        ):
            xT = iopool.tile([128, 8, N], b16)
            wqkv = iopool.tile([128, 8, 1536], b16)
            wp = iopool.tile([128, 4, C], b16)
            neglam = iopool.tile([128, 1], f32)
            # qkvT chunk j=2*hp+t: partitions 0-63 even head d0..63,
            # 64-127 odd head d0..63
            qkvT = iopool.tile([128, 8, N], b16)
            # V in [keys, channels] layout; col 64 of each head = ones
            vsb = iopool.tile([128, 8, 8, 65], b16)
            # transposed attention out for proj: [ch-part, hp, n]
            oT = iopool.tile([128, 4, N], b16)
            ident = iopool.tile([128, 128], b16)
            if debug:
                dbg_oc_t = iopool.tile([128, 8, 4, 2, 64], b16)

            warm = iopool.tile([128, 256], b16)
            nc.gpsimd.memset(warm[:], 0.5)
            nc.gpsimd.memset(vsb[:, :, :, 64:65], 1.0)

            # ---------------- DMA in (strict need order) ----------------
            # The shared DMA slot grants waiting transfers in ARBITRARY
            # order, so a late-need transfer that queues early can starve a
            # critical one.  Put everything on the sync queue in exact need
            # order - its ~1.3us per-issue pacing self-throttles - except
            # w[j0,j1] which rides the otherwise-empty scalar queue.
            nc.scalar.dma_start(wqkv[:, :, 0:256], wqkv_d[:, :, 0:256])
            for cc2 in range(4):
                nc.sync.dma_start(
                    xT[:, 2 * cc2 : 2 * cc2 + 2, 0:512],
                    x_d[:, 2 * cc2 : 2 * cc2 + 2, 0:512],
                )
            for h4 in range(2):
                nc.sync.dma_start(
                    xT[:, 4 * h4 : 4 * h4 + 4, 512:1024],
                    x_d[:, 4 * h4 : 4 * h4 + 4, 512:1024],
                )
            nc.sync.dma_start(wqkv[:, :, 1024:1280], wqkv_d[:, :, 1024:1280])
            nc.sync.dma_start(wqkv[:, :, 1280:1536], wqkv_d[:, :, 1280:1536])
            nc.gpsimd.dma_start(neglam[:], neglam_d[:])
            for hp in range(1, 4):
                c0 = hp * 256
                nc.sync.dma_start(wqkv[:, :, c0 : c0 + 256], wqkv_d[:, :, c0 : c0 + 256])
            nc.sync.dma_start(wp[:], wp_d[:])
            nc.sync.dma_start(ident[:], ident_d[:])

            # PE warm-up during the input DMAs: the pstate model needs ~3us
            # of continuous execution to reach full clock, so burn it on
            # dummy matmuls into a single scratch PSUM tile (one slot alloc
            # so the pP ring is not serialized)
            wps = pP.tile([128, 512], f32, tag="p", name="warm")
            for w in range(10):
                nc.tensor.matmul(
                    wps[0:1, 0:256], warm[:, 0:1], warm[:], start=True, stop=True
                )

            # ---------------- filler units ------------------------------
            def emit_qkv_unit(hp, t, nh, pool=None):
                # one [128,512] q/k projection chunk -> qkvT[:, 2hp+t, nh]
                j = 2 * hp + t
                pool = pool or pP
                ps = pool.tile([128, 512], f32, tag=pool.name[1].lower(),
                               name=f"qkv{j}_{nh}")
                for cc in range(8):
                    nc.tensor.matmul(
                        ps[:],
                        wqkv[:, cc, j * 128 : (j + 1) * 128],
                        xT[:, cc, nh * 512 : (nh + 1) * 512],
                        start=(cc == 0),
                        stop=(cc == 7),
                    )
                nc.vector.tensor_copy(qkvT[:, j, nh * 512 : (nh + 1) * 512], ps[:])

            def emit_v_unit(mc):
                # v -> [keys, channels] layout (operands swapped); DVE evac
                ps = pP.tile([128, 512], f32, tag="p", name=f"v{mc}")
                for cc in range(8):
                    nc.tensor.matmul(
                        ps[:],
                        xT[:, cc, mc * 128 : (mc + 1) * 128],
                        wqkv[:, cc, 1024:1536],
                        start=(cc == 0),
                        stop=(cc == 7),
                    )
                nc.vector.tensor_copy(
                    vsb[:, mc, :, 0:64], ps.rearrange("p (g d) -> p g d", g=8)
                )

            def emit_proj_unit(ncc, jh, pool, on_act=False):
                # out[ncc n-chunk, jh 512 out-ch] = oT.T @ wp, K=512 (4 ci)
                # tail units evacuate on ACT (idle once the exps are done)
                ps = pool.tile([128, 512], f32, tag=pool.name[1].lower(),
                               name=f"proj{ncc}_{jh}")
                for ci in range(4):
                    nc.tensor.matmul(
                        ps[:],
                        oT[:, ci, ncc * 128 : (ncc + 1) * 128],
                        wp[:, ci, jh * 512 : (jh + 1) * 512],
                        start=(ci == 0),
                        stop=(ci == 3),
                    )
                osb = wpool.tile([128, 512], b16, tag="osb")
                if on_act:
                    # tail: ACT evac + scalar-queue DMA keeps the sync queue
                    # free for the final transposes' waits
                    nc.scalar.copy(osb[:], ps[:])
                    nc.scalar.dma_start(out_d[ncc, :, jh * 512 : (jh + 1) * 512], osb[:])
                else:
                    nc.vector.tensor_copy(osb[:], ps[:])
                    nc.sync.dma_start(out_d[ncc, :, jh * 512 : (jh + 1) * 512], osb[:])

            # ---------------- attention pieces --------------------------
            def emit_scores_exp(hp, nh, mc):
                # 4 score matmuls + 2 exps; returns e tiles [g0, g1]
                cur = []
                for g in range(2):
                    s_ps = pS.tile([128, 2, 512], f32, tag="s")
                    for i in range(2):
                        rg = RG[2 * g + i]
                        nc.tensor.matmul(
                            s_ps[:, i, :],
                            qkvT[
                                32 * rg : 32 * rg + 32,
                                2 * hp + 1,
                                mc * 128 : (mc + 1) * 128,
                            ],
                            qkvT[
                                32 * rg : 32 * rg + 32,
                                2 * hp,
                                nh * 512 : (nh + 1) * 512,
                            ],
                            start=True,
                            stop=True,
                            tile_position=(32 * rg, 0),
                        )
                    e_sb = epool.tile([128, 2, 512], b16, tag="e")
                    nc.scalar.activation(e_sb[:], s_ps[:], Exp, scale=0.125)
                    cur.append(e_sb)
                return cur

            def emit_pv(hp, etiles, o, mc):
                # stationary-E PV into the 3-bank packed tile o [128,3,512]
                # chunk g=4*ci+nsub at (bank g//7, col (g%7)*65); start=True
                # zeroes the whole bank so only its first chunk may set it
                for ci in range(4):
                    par = ci % 2
                    for nsub in range(4):
                        g = 4 * ci + nsub
                        b, sl = GB[g]
                        nc.tensor.matmul(
                            o[:, b, sl * 65 : sl * 65 + 65],
                            etiles[ci // 2][:, par, nsub * 128 : (nsub + 1) * 128],
                            vsb[:, mc, 2 * hp + par, :],
                            start=(mc == 0 and g in G_START),
                            stop=(mc == 7 and g in G_STOP),
                            skip_group_check=True,
                        )

            def emit_oraw(o):
                # bulk-evacuate the packed accumulator to SBUF (3 copies,
                # one per bank) so the PSUM tile is released fast - the
                # normalization then runs off the critical path from SBUF
                oraw = wpool.tile([128, 16, 65], f32, tag="oraw")
                orv = oraw.rearrange("p g c -> p (g c)")
                for b, cnt in ((0, 7), (1, 7), (2, 2)):
                    nc.vector.tensor_copy(
                        orv[:, b * 455 : b * 455 + cnt * 65], o[:, b, 0 : cnt * 65]
                    )
                return oraw

            def emit_recips(oraw):
                # per-partition reciprocals of the stride-65 denominators
                r = wpool.tile([128, 16, 1], f32, tag="r")
                nc.vector.reciprocal(r[:], oraw[:, :, 64:65])
                # -lam fold on the h2 chunks (g 8..15)
                nc.vector.tensor_scalar_mul(r[:, 8:16], r[:, 8:16], neglam[:])
                return r

            def emit_combine_chunk(hp, oraw, r, oc, nsub):
                # one nsub (128 n cols): 2 parities, fused on DVE, all SBUF
                for par in range(2):
                    g1 = 4 * par + nsub
                    g2 = 8 + 4 * par + nsub
                    t = wpool.tile([128, 64], f32, tag=f"t{par}")
                    nc.vector.tensor_scalar_mul(
                        t[:], oraw[:, g2, 0:64], r[:, g2]
                    )
                    nc.vector.scalar_tensor_tensor(
                        oc[:, nsub, par, :],
                        oraw[:, g1, 0:64],
                        r[:, g1],
                        t[:],
                        MUL,
                        ADD,
                    )

            def emit_combine(hp, nh, o):
                oraw = emit_oraw(o)
                r = emit_recips(oraw)
                oc = ocpool.tile([128, 4, 2, 64], b16, tag="oc")
                for nsub in range(4):
                    emit_combine_chunk(hp, oraw, r, oc, nsub)
                if debug:
                    nc.vector.tensor_copy(dbg_oc_t[:, 2 * hp + nh], oc[:])
                return oc

            def emit_transpose(hp, nh, oc, nsub):
                # oc[128 n, par, 64ch] -> oT[128 ch, n] via DMA xbar
                n0 = nh * 512 + nsub * 128
                nc.sync.dma_start_transpose(oT[:, hp, n0 : n0 + 128], oc[:, nsub])

            def emit_transpose_pe(hp, nh, oc, nsub):
                # final-sweep transpose on PE (PSUM is free, and the ~2.4us
                # DMA-transpose latency would sit on the critical tail)
                n0 = nh * 512 + nsub * 128
                trb = pS.tile([128, 128], b16, tag="s", name="trb")
                nc.tensor.matmul(trb[:], oc[:, nsub], ident[:], is_transpose=True)
                nc.scalar.copy(oT[:, hp, n0 : n0 + 128], trb[:])

            # ---------------- filler schedule ---------------------------
            filler = {}

            def add(slot, fn, *args):
                filler.setdefault(slot, []).append((fn, args))

            # sweep 0 (hp0,nh0): V units + rest of hp0 qkv
            add((0, 0), emit_v_unit, 0)
            add((0, 1), emit_v_unit, 1)
            add((0, 2), emit_qkv_unit, 0, 1, 1)  # k hp0 keys 512-1023 (mc4+)
            add((0, 3), emit_v_unit, 2)
            add((0, 4), emit_v_unit, 3)
            add((0, 5), emit_qkv_unit, 0, 0, 1)  # q hp0 n 512+ (sweep 1)
            add((0, 6), emit_v_unit, 4)
            add((0, 7), emit_v_unit, 5)
            add((0, 7), emit_v_unit, 6)
            add((0, 7), emit_v_unit, 7)
            # sweep 1: hp1 qkv (needed by sweep 2)
            add((1, 1), emit_qkv_unit, 1, 1, 0)
            add((1, 3), emit_qkv_unit, 1, 1, 1)
            add((1, 5), emit_qkv_unit, 1, 0, 0)
            add((1, 7), emit_qkv_unit, 1, 0, 1)
            # sweeps 2-3: hp2 (needed by sweep 4)
            add((2, 2), emit_qkv_unit, 2, 1, 0)
            add((2, 6), emit_qkv_unit, 2, 1, 1)
            add((3, 2), emit_qkv_unit, 2, 0, 0)
            add((3, 6), emit_qkv_unit, 2, 0, 1)
            # sweeps 4-5: hp3 (needed by sweep 6)
            add((4, 2), emit_qkv_unit, 3, 1, 0)
            add((4, 6), emit_qkv_unit, 3, 1, 1)
            add((5, 2), emit_qkv_unit, 3, 0, 0)
            add((5, 6), emit_qkv_unit, 3, 0, 1)
            # sweep 7: proj units for n-half 0; the rest are held back to
            # fill the final-combine window (fillers here starve the exps)
            add((7, 2), emit_proj_unit, 0, 0, pP)
            add((7, 4), emit_proj_unit, 0, 1, pP)
            add((7, 6), emit_proj_unit, 1, 0, pP)

            # qkv for the first sweep must precede it; q rides a free pS
            # slot so its matmuls don't wait on the k unit's evacuation
            emit_qkv_unit(0, 1, 0)  # k hp0 keys 0-511
            emit_qkv_unit(0, 0, 0, pool=pS)  # q hp0 n 0-511

            # ---------------- main pipeline -----------------------------
            # software-pipelined across sweep boundaries: the tail PVs of
            # sweep s are emitted after sweep s+1's first scores/exp so the
            # next exp is never behind them on PE; combine lands at mc==1
            pending = None  # (hp, nh, o_tile)
            tailpv = None  # (hp, edeque, o_tile)
            for s in range(8):
                hp, nh = s // 2, s % 2
                o = None
                edeque = []
                for mc in range(8):
                    edeque.append(emit_scores_exp(hp, nh, mc))
                    if mc == 0 and tailpv is not None:
                        for mcp in range(8 - LAG, 8):
                            emit_pv(tailpv[0], tailpv[1][mcp], tailpv[2], mcp)
                        tailpv = None
                    for fn, args in filler.get((s, mc), ()):
                        fn(*args)
                    if mc == 1 and pending is not None:
                        oc = emit_combine(*pending)
                        for nsub in range(4):
                            emit_transpose(pending[0], pending[1], oc, nsub)
                        pending = None
                    if mc >= LAG:
                        if o is None:
                            o = pO.tile([128, 3, 512], f32, tag="o", name=f"o{s}")
                        emit_pv(hp, edeque[mc - LAG], o, mc - LAG)
                pending = (hp, nh, o)
                tailpv = (hp, edeque, o)

            # final sweep: tail PVs, then chunked combine -> transpose ->
            # proj pipeline; two held-back n-half-0 proj units keep PE warm
            hp, nh, o = pending
            for mcp in range(8 - LAG, 8):
                emit_pv(hp, tailpv[1][mcp], o, mcp)
            oraw = emit_oraw(o)
            r = emit_recips(oraw)
            # held-back n-half-0 proj units fill the combine window on PE
            emit_proj_unit(1, 1, pS, on_act=True)
            emit_proj_unit(2, 0, pP, on_act=True)
            emit_proj_unit(2, 1, pS, on_act=True)
            emit_proj_unit(3, 0, pP, on_act=True)
            emit_proj_unit(3, 1, pS, on_act=True)
            oc = ocpool.tile([128, 4, 2, 64], b16, tag="oc")
            for nsub in range(4):
                emit_combine_chunk(hp, oraw, r, oc, nsub)
                emit_transpose_pe(hp, nh, oc, nsub)
                emit_proj_unit(4 + nsub, 0, pP if nsub % 2 else pS, on_act=True)
                emit_proj_unit(4 + nsub, 1, pS if nsub % 2 else pP, on_act=True)
            if debug:
                nc.vector.tensor_copy(dbg_oc_t[:, 2 * hp + nh], oc[:])

            if debug:
                nc.sync.dma_start(dbg_qkvT[:], qkvT[:])
                nc.sync.dma_start(dbg_vsb[:], vsb[:])
                nc.sync.dma_start(dbg_oc[:], dbg_oc_t[:])
                nc.sync.dma_start(dbg_oT[:], oT[:])

    nc.compile()
    _PROG_CACHE[key] = nc
    return nc


def _prep_core_inputs(x, W_qkv, W_proj, neg_lam):
    """Host-side shard prep. Returns in_maps for the 8 cores."""
    W4 = np.asarray(W_qkv, np.float32).reshape(3, H, HD, C)
    in_maps = []
    for core in range(8):
        b, hg = divmod(core, 2)
        xT = (
            np.ascontiguousarray(np.asarray(x[b], np.float32).T)
            .reshape(8, 128, N)
            .transpose(1, 0, 2)
            .astype(BF16)
        )
        wsl = W4[:, hg * 8 : (hg + 1) * 8]  # [3, 8 heads, 64, 1024]
        # columns [t(3), head(8), d(64)]; reorder q/k to j'=2*hp+t blocks
        Wcols = np.ascontiguousarray(wsl.transpose(3, 0, 1, 2).reshape(C, 1536))
        Wnew = np.empty_like(Wcols)
        for hp in range(4):
            for t in range(2):
                src = t * 512 + hp * 128
                dst = (2 * hp + t) * 128
                Wnew[:, dst : dst + 128] = Wcols[:, src : src + 128]
        Wnew[:, 1024:1536] = Wcols[:, 1024:1536]
        wqkv = Wnew.reshape(8, 128, 1536).transpose(1, 0, 2).astype(BF16)
        wp = (
            np.ascontiguousarray(
                np.asarray(W_proj, np.float32)[:, hg * 512 : (hg + 1) * 512].T
            )
            .reshape(4, 128, C)
            .transpose(1, 0, 2)
            .astype(BF16)
        )
        in_maps.append(
            {
                "xT": np.ascontiguousarray(xT),
                "wqkv": np.ascontiguousarray(wqkv),
                "wp": np.ascontiguousarray(wp),
                "neglam": np.full((128, 1), neg_lam, np.float32),
                "ident": np.eye(128, dtype=np.float32).astype(BF16),
            }
        )
    return in_maps


def kernel(x, W_qkv, W_proj, b_proj, lambda_q1, lambda_k1, lambda_q2, lambda_k2):
    from concourse.bass_utils import run_bass_kernel_spmd

    lq1 = np.asarray(lambda_q1, np.float64)
    lk1 = np.asarray(lambda_k1, np.float64)
    lq2 = np.asarray(lambda_q2, np.float64)
    lk2 = np.asarray(lambda_k2, np.float64)
    lam = float(np.mean(np.exp(lq1 * lk1) - np.exp(lq2 * lk2) + LAMBDA_INIT))

    nc = _build_program()
    in_maps = _prep_core_inputs(x, W_qkv, W_proj, -lam)
    res = run_bass_kernel_spmd(nc, in_maps, core_ids=list(range(8)))
    _PROG_CACHE["last_result"] = res

    bp = np.asarray(b_proj, np.float32)
    out = np.empty((B, N, C), np.float32)
    for b in range(B):
        p0 = res.results[2 * b]["out"].astype(np.float32).reshape(N, C)
        p1 = res.results[2 * b + 1]["out"].astype(np.float32).reshape(N, C)
        out[b] = p0 + p1 + bp[None, :]
    return out
